# revision 1
# baseline (speedup 1.0000x reference)
"""BinaryLinear (binarized nn.Linear) on 8 Trainium2 NeuronCores.

Reference op:
    alpha = mean(|W|, axis=1)                # per-output-row scale
    BW    = sign(W) * alpha                  # sign(0) := +1
    Y     = einsum('bsi,oi->bso', X, BW) + bias

Distribution: data-parallel over the batch dim (8 batches -> 1 per core).
Each core receives its batch slice of X pre-transposed (xT = [in, tok]),
the full weight in both layouts (wT = [in, out] for the matmul stationary
operand, w = [out, in] for the per-row alpha reduction), and bias. Each
core computes the full [tok, out] output for its batch element (stored
transposed as [out, tok]); the host transposes back and stacks.

On-device per core:
  - sign half-trick: s = (w >= 0) - 0.5 in {+0.5, -0.5} (one DVE op, exact
    in every dtype); the missing x2 is folded into alpha2 = 2*mean|W|.
  - alpha: DVE abs-accumulate reduce over natural-layout weight rows.
  - matmul: fp32r (full-rate fp32 PE mode), K=2048 accumulated in PSUM.
    Out-chunks are processed in PAIRS with the k-chunk loop OUTERMOST so
    each arriving x-chunk unblocks 8 matmuls (all 8 PSUM banks) -- this
    hides the initial 16 MiB x load behind PE work.
  - epilogue: one ScalarE activation per psum tile:
    Identity(psum*alpha2 + bias), then DMA out.
"""

import os

import numpy as np

B, T, K, O = 8, 2048, 2048, 2048  # batch, tokens, in_features, out_features
P = 128          # SBUF partitions
KC = K // P      # 16 k-chunks
OC = O // P      # 16 out-chunks
TN = 512         # moving free-dim per matmul (fp32 max)
TT = T // TN     # 4 token tiles

N_CORES = 8

# Stashed by kernel() for test harnesses: BassKernelResults of the last run.
last_results = None

_cached_nc = None


def _build_program():
    global _cached_nc
    if _cached_nc is not None:
        return _cached_nc

    import concourse.tile as tile
    from concourse import bacc, bass_isa, mybir

    F32 = mybir.dt.float32
    F32R = mybir.dt.float32r
    IDENT = mybir.ActivationFunctionType.Identity
    ALU = mybir.AluOpType
    AX = mybir.AxisListType

    nc = bacc.Bacc("TRN2", target_bir_lowering=False, debug=False,
                   num_devices=N_CORES)

    # x arrives pre-transposed and is consumed as the (reduced-precision)
    # f32r moving operand directly -- no on-chip cast pass.
    xT = nc.dram_tensor("xT", [K, T], F32R, kind="ExternalInput").ap()
    wT = nc.dram_tensor("wT", [K, O], F32, kind="ExternalInput").ap()
    w = nc.dram_tensor("w", [O, K], F32, kind="ExternalInput").ap()
    b = nc.dram_tensor("b", [O], F32, kind="ExternalInput").ap()
    yT = nc.dram_tensor("yT", [O, T], F32, kind="ExternalOutput").ap()

    xT_r = xT.rearrange("(c p) t -> p c t", p=P)
    wT_r = wT.rearrange("(c p) o -> p c o", p=P)

    with tile.TileContext(nc) as tc:
        with (
            tc.tile_pool(name="xpool", bufs=1) as xpool,
            tc.tile_pool(name="wpool", bufs=2) as wpool,
            tc.tile_pool(name="spool", bufs=3) as spool,
            tc.tile_pool(name="npool", bufs=2) as npool,
            tc.tile_pool(name="apool", bufs=4) as apool,
            tc.tile_pool(name="opool", bufs=3) as opool,
            tc.tile_pool(name="const", bufs=1) as const,
            tc.tile_pool(name="psum", bufs=8, space="PSUM") as psum,
        ):
            def sign_prep(o):
                """Load + binarize the stationary operand for out-chunk o."""
                wraw = wpool.tile([P, KC, P], F32, tag="wraw",
                                  name=f"wraw{o}")
                nc.sync.dma_start(out=wraw, in_=wT_r[:, :, o * P:(o + 1) * P])
                sw = spool.tile([P, KC, P], F32R, tag="sw", name=f"sw{o}")
                nc.vector.tensor_scalar(sw, wraw, 0.0, 0.5,
                                        op0=ALU.is_ge, op1=ALU.subtract)
                return sw

            def alpha_prep(o):
                """alpha2 = 2*mean|W_row| from the natural-layout rows."""
                wn = npool.tile([P, K], F32, tag="wn", name=f"wn{o}")
                nc.sync.dma_start(out=wn, in_=w[o * P:(o + 1) * P, :])
                asum = apool.tile([P, 1], F32, tag="asum", name=f"as{o}")
                nc.vector.tensor_reduce(asum, wn, axis=AX.X, op=ALU.add,
                                        apply_absolute_value=True)
                alpha2 = apool.tile([P, 1], F32, tag="alpha2", name=f"al{o}")
                nc.vector.tensor_scalar_mul(alpha2, asum, 2.0 / K)
                return alpha2

            def weight_prep(o):
                return sign_prep(o), alpha_prep(o)

            # only pair-0 weights go ahead of the x stream
            prepped = {0: weight_prep(0), 1: weight_prep(1)}

            # resident x: 16 chunk tiles [128, 2048] f32r (i on partitions)
            x_tiles = []
            bias_sb = None
            for c in range(KC):
                xt = xpool.tile([P, T], F32R, tag=f"x{c}")
                nc.sync.dma_start(out=xt, in_=xT_r[:, c, :])
                x_tiles.append(xt)
                if c == 3:
                    # bias: epilogue-only, tiny [128,16]
                    bias_sb = const.tile([P, OC], F32)
                    nc.sync.dma_start(out=bias_sb,
                                      in_=b.rearrange("(c p) -> p c", p=P))

            def mm_group(ps_t, sw, t, c_lo, c_hi):
                for c in range(c_lo, c_hi):
                    nc.tensor.matmul(
                        ps_t, lhsT=sw[:, c, :],
                        rhs=x_tiles[c][:, t * TN:(t + 1) * TN],
                        start=(c == c_lo), stop=(c == c_hi - 1))

            def epilogue(ps_t, o, t, a2, name):
                ot = opool.tile([P, TN], F32, tag="ot", name=name)
                nc.scalar.activation(ot, ps_t, IDENT,
                                     bias=bias_sb[:, o:o + 1], scale=a2)
                # issue output DMAs on the ACT HW-DGE ring: the SP ring's
                # in-order issue stream must stay pure loads, else weight
                # prefetch DMAs queue behind epilogue-gated stores
                nc.scalar.dma_start(
                    out=yT[o * P:(o + 1) * P, t * TN:(t + 1) * TN], in_=ot)

            for pair in range(OC // 2):
                o0, o1 = 2 * pair, 2 * pair + 1
                pair_w = [prepped.pop(o0), prepped.pop(o1)]
                ps = [psum.tile([P, TN], F32, tag="ps", name=f"ps{pair}_{i}")
                      for i in range(8)]

                if pair < 2:
                    # x still streaming in: k-chunk outermost so every
                    # arriving x chunk unblocks 8 matmuls (all psum banks)
                    for c in range(KC):
                        for j in range(2):
                            sw = pair_w[j][0]
                            for t in range(TT):
                                nc.tensor.matmul(
                                    ps[j * TT + t],
                                    lhsT=sw[:, c, :],
                                    rhs=x_tiles[c][:, t * TN:(t + 1) * TN],
                                    start=(c == 0),
                                    stop=(c == KC - 1),
                                )
                    for j in range(2):
                        for t in range(TT):
                            epilogue(ps[j * TT + t], (o0, o1)[j], t,
                                     pair_w[j][1], f"ot{pair}_{j}_{t}")
                else:
                    # steady state: one psum group at a time so groups finish
                    # staggered -- banks free incrementally and epilogues
                    # overlap the next group's matmuls
                    for j in range(2):
                        for t in range(TT):
                            mm_group(ps[j * TT + t], pair_w[j][0], t, 0, KC)
                            epilogue(ps[j * TT + t], (o0, o1)[j], t,
                                     pair_w[j][1], f"ot{pair}_{j}_{t}")

                # prefetch next pair's weights (emitted after this pair's
                # matmuls so the DMAs queue behind the x chunks)
                if pair + 1 < OC // 2:
                    prepped[2 * pair + 2] = weight_prep(2 * pair + 2)
                    prepped[2 * pair + 3] = weight_prep(2 * pair + 3)

    nc.compile()
    _cached_nc = nc
    return nc


def _make_in_maps(x, weight, bias):
    wT = np.ascontiguousarray(weight.T)
    w = np.ascontiguousarray(weight)
    b = np.ascontiguousarray(bias)
    in_maps = []
    for core in range(N_CORES):
        xb = np.ascontiguousarray(x[core].T)  # [in, tok]
        in_maps.append({"xT": xb, "wT": wT, "w": w, "b": b})
    return in_maps


def _setup_trace_hooks():
    """Provide the antenv.axon_hooks NTFF hook missing from this image and
    skip the artifact bucket upload so trace=True works locally."""
    import sys
    import types

    try:
        from antenv.axon_hooks import get_axon_ntff_profile_hook  # noqa: F401
    except ImportError:
        mod = types.ModuleType("antenv.axon_hooks")
        _h = [None]
        mod.set_axon_ntff_profile_hook = lambda h: _h.__setitem__(0, h)
        mod.get_axon_ntff_profile_hook = lambda: _h[0]
        sys.modules["antenv.axon_hooks"] = mod
        import antenv

        antenv.axon_hooks = mod
        from trn_agent_boot.trn_boot import _ntff_profile_via_ctypes

        mod.set_axon_ntff_profile_hook(
            _ntff_profile_via_ctypes("/opt/axon/libaxon_pjrt.so"))

    import concourse.bass_utils as bu

    bu.upload_artifacts = lambda tmpdir: f"local://{tmpdir}"


def kernel(x: np.ndarray, weight: np.ndarray, bias: np.ndarray) -> np.ndarray:
    global last_results
    from concourse.bass_utils import run_bass_kernel_spmd

    x = np.asarray(x, dtype=np.float32)
    weight = np.asarray(weight, dtype=np.float32)
    bias = np.asarray(bias, dtype=np.float32)

    nc = _build_program()
    in_maps = _make_in_maps(x, weight, bias)
    trace = bool(int(os.environ.get("KERNEL_TRACE", "0")))
    trace_cores = None
    if trace:
        _setup_trace_hooks()
        tc_env = os.environ.get("KERNEL_TRACE_CORES", "")
        if tc_env:
            trace_cores = [int(c) for c in tc_env.split(",")]
    res = run_bass_kernel_spmd(nc, in_maps, list(range(N_CORES)), trace=trace,
                               trace_cores=trace_cores)
    last_results = res

    out = np.empty((B, T, O), dtype=np.float32)
    for core in range(N_CORES):
        out[core] = res.results[core]["yT"].T
    return out



# revision 2
# speedup vs baseline: 1.0372x; 1.0372x over previous
"""BinaryLinear (binarized nn.Linear) on 8 Trainium2 NeuronCores.

Reference op:
    alpha = mean(|W|, axis=1)                # per-output-row scale
    BW    = sign(W) * alpha                  # sign(0) := +1
    Y     = einsum('bsi,oi->bso', X, BW) + bias

Distribution: data-parallel over the batch dim (8 batches -> 1 per core).
Each core receives its batch slice of X pre-transposed and cast to bf16
(xT = [in, tok]), the full weight in both layouts as bf16 (wT = [in, out]
for the matmul stationary operand, w = [out, in] for the per-row alpha
reduction), and bias f32. Each core computes the full [tok, out] output
for its batch element (stored transposed as [out, tok], bf16); the host
casts back to f32, transposes and stacks.

Numerics: binarized weights are exactly +-0.5 in bf16 (the missing x2 is
folded into alpha2 = 2*mean|W|), so the only quantization is x->bf16 and
the bf16 output store: ~0.2% rel error vs the 2e-2 gate.

On-device per core:
  - sign half-trick: s = (w >= 0) - 0.5 in {+0.5, -0.5} (one DVE op).
  - alpha: DVE abs-accumulate reduce over natural-layout bf16 weight rows
    into f32.
  - matmul: bf16 (full-rate PE + FWL weight loads), K=2048 accumulated in
    PSUM f32. Warmup: pair-0 out-chunks run with the k-chunk loop
    OUTERMOST so each arriving 512 KiB x-chunk unblocks 8 matmuls (all 8
    PSUM banks); bf16 chunk DMA (1.4us) < 8 MMs (1.8us) so the PE never
    starves once the first chunk lands.
  - DMA emission order on the in-order SP ring: pair-0 sign source first,
    then the 16 x chunks, then alphas + later pairs' weights - this puts
    the first matmul ~4us in instead of waiting on all weight prep.
  - epilogue: one ScalarE activation per psum tile:
    Identity(psum*alpha2 + bias) -> bf16, then DMA out on the ACT HW-DGE
    ring (keeps the SP ring pure loads).
"""

import os

import numpy as np

B, T, K, O = 8, 2048, 2048, 2048  # batch, tokens, in_features, out_features
P = 128          # SBUF partitions
KC = K // P      # 16 k-chunks
OC = O // P      # 16 out-chunks
TN = 512         # moving free-dim per matmul
TT = T // TN     # 4 token tiles

N_CORES = 8

# Stashed by kernel() for test harnesses: BassKernelResults of the last run.
last_results = None

_cached_nc = None


def _build_program():
    global _cached_nc
    if _cached_nc is not None:
        return _cached_nc

    import concourse.tile as tile
    from concourse import bacc, bass_isa, mybir

    F32 = mybir.dt.float32
    BF16 = mybir.dt.bfloat16
    IDENT = mybir.ActivationFunctionType.Identity
    ALU = mybir.AluOpType
    AX = mybir.AxisListType

    nc = bacc.Bacc("TRN2", target_bir_lowering=False, debug=False,
                   num_devices=N_CORES)

    xT = nc.dram_tensor("xT", [K, T], BF16, kind="ExternalInput").ap()
    wT = nc.dram_tensor("wT", [K, O], BF16, kind="ExternalInput").ap()
    w = nc.dram_tensor("w", [O, K], BF16, kind="ExternalInput").ap()
    b = nc.dram_tensor("b", [O], F32, kind="ExternalInput").ap()
    yT = nc.dram_tensor("yT", [O, T], BF16, kind="ExternalOutput").ap()

    xT_r = xT.rearrange("(c p) t -> p c t", p=P)
    wT_r = wT.rearrange("(c p) o -> p c o", p=P)

    with tile.TileContext(nc) as tc:
        with (
            tc.tile_pool(name="xpool", bufs=1) as xpool,
            tc.tile_pool(name="wpool", bufs=2) as wpool,
            tc.tile_pool(name="spool", bufs=4) as spool,
            tc.tile_pool(name="npool", bufs=2) as npool,
            tc.tile_pool(name="apool", bufs=6) as apool,
            tc.tile_pool(name="opool", bufs=3) as opool,
            tc.tile_pool(name="const", bufs=1) as const,
            tc.tile_pool(name="psum", bufs=8, space="PSUM") as psum,
        ):
            def sign_prep(o):
                """Load + binarize the stationary operand for out-chunk o."""
                wraw = wpool.tile([P, KC, P], BF16, tag="wraw",
                                  name=f"wraw{o}")
                nc.sync.dma_start(out=wraw, in_=wT_r[:, :, o * P:(o + 1) * P])
                sw = spool.tile([P, KC, P], BF16, tag="sw", name=f"sw{o}")
                nc.vector.tensor_scalar(sw, wraw, 0.0, 0.5,
                                        op0=ALU.is_ge, op1=ALU.subtract)
                return sw

            def alpha_prep(o):
                """alpha2 = 2*mean|W_row| from the natural-layout rows."""
                wn = npool.tile([P, K], BF16, tag="wn", name=f"wn{o}")
                nc.sync.dma_start(out=wn, in_=w[o * P:(o + 1) * P, :])
                asum = apool.tile([P, 1], F32, tag="asum", name=f"as{o}")
                nc.vector.tensor_reduce(asum, wn, axis=AX.X, op=ALU.add,
                                        apply_absolute_value=True)
                alpha2 = apool.tile([P, 1], F32, tag="alpha2", name=f"al{o}")
                nc.vector.tensor_scalar_mul(alpha2, asum, 2.0 / K)
                return alpha2

            def weight_prep(o):
                return sign_prep(o), alpha_prep(o)

            # pair-0 sign source goes ahead of the x stream (the first
            # matmuls need it); everything else queues behind x
            sw0 = sign_prep(0)
            sw1 = sign_prep(1)

            # resident x: 16 chunk tiles [128, 2048] bf16 (i on partitions)
            x_tiles = []
            bias_sb = None
            for c in range(KC):
                xt = xpool.tile([P, T], BF16, tag=f"x{c}")
                nc.sync.dma_start(out=xt, in_=xT_r[:, c, :])
                x_tiles.append(xt)
                if c == 1:
                    # bias: epilogue-only, tiny [128,16]
                    bias_sb = const.tile([P, OC], F32)
                    nc.sync.dma_start(out=bias_sb,
                                      in_=b.rearrange("(c p) -> p c", p=P))

            # alphas for pair 0 (needed by its epilogues ~30us in), then
            # pair-1 weights (needed when steady state starts)
            a0 = alpha_prep(0)
            a1 = alpha_prep(1)
            prepped = {0: (sw0, a0), 1: (sw1, a1),
                       2: weight_prep(2), 3: weight_prep(3)}

            def mm_group(ps_t, sw, t, c_lo, c_hi):
                for c in range(c_lo, c_hi):
                    nc.tensor.matmul(
                        ps_t, lhsT=sw[:, c, :],
                        rhs=x_tiles[c][:, t * TN:(t + 1) * TN],
                        start=(c == c_lo), stop=(c == c_hi - 1))

            def epilogue(ps_t, o, t, a2, name):
                ot = opool.tile([P, TN], BF16, tag="ot", name=name)
                nc.scalar.activation(ot, ps_t, IDENT,
                                     bias=bias_sb[:, o:o + 1], scale=a2)
                # issue output DMAs on the ACT HW-DGE ring: the SP ring's
                # in-order issue stream must stay pure loads, else weight
                # prefetch DMAs queue behind epilogue-gated stores
                nc.scalar.dma_start(
                    out=yT[o * P:(o + 1) * P, t * TN:(t + 1) * TN], in_=ot)

            for pair in range(OC // 2):
                o0, o1 = 2 * pair, 2 * pair + 1
                pair_w = [prepped.pop(o0), prepped.pop(o1)]
                ps = [psum.tile([P, TN], F32, tag="ps", name=f"ps{pair}_{i}")
                      for i in range(8)]

                if pair == 0:
                    # x still streaming in: k-chunk outermost so every
                    # arriving x chunk unblocks 8 matmuls (all psum banks)
                    for c in range(KC):
                        for j in range(2):
                            sw = pair_w[j][0]
                            for t in range(TT):
                                nc.tensor.matmul(
                                    ps[j * TT + t],
                                    lhsT=sw[:, c, :],
                                    rhs=x_tiles[c][:, t * TN:(t + 1) * TN],
                                    start=(c == 0),
                                    stop=(c == KC - 1),
                                )
                    for j in range(2):
                        for t in range(TT):
                            epilogue(ps[j * TT + t], (o0, o1)[j], t,
                                     pair_w[j][1], f"ot{pair}_{j}_{t}")
                else:
                    # steady state: one psum group at a time so groups finish
                    # staggered -- banks free incrementally and epilogues
                    # overlap the next group's matmuls
                    for j in range(2):
                        for t in range(TT):
                            mm_group(ps[j * TT + t], pair_w[j][0], t, 0, KC)
                            epilogue(ps[j * TT + t], (o0, o1)[j], t,
                                     pair_w[j][1], f"ot{pair}_{j}_{t}")

                # prefetch weights two pairs out (pair 0 and 1 were loaded
                # up front); emitted after this pair's matmuls so the DMAs
                # queue behind the x chunks on the in-order SP ring
                nxt = 2 * pair + 4
                if nxt < OC:
                    prepped[nxt] = weight_prep(nxt)
                    prepped[nxt + 1] = weight_prep(nxt + 1)

    nc.compile()
    _cached_nc = nc
    return nc


def _make_in_maps(x, weight, bias):
    import ml_dtypes

    bf16 = ml_dtypes.bfloat16
    wT = np.ascontiguousarray(weight.T).astype(bf16)
    w = np.ascontiguousarray(weight).astype(bf16)
    b = np.ascontiguousarray(bias)
    in_maps = []
    for core in range(N_CORES):
        xb = np.ascontiguousarray(x[core].T).astype(bf16)  # [in, tok]
        in_maps.append({"xT": xb, "wT": wT, "w": w, "b": b})
    return in_maps


def _setup_trace_hooks():
    """Provide the antenv.axon_hooks NTFF hook missing from this image and
    skip the artifact bucket upload so trace=True works locally."""
    import sys
    import types

    try:
        from antenv.axon_hooks import get_axon_ntff_profile_hook  # noqa: F401
    except ImportError:
        mod = types.ModuleType("antenv.axon_hooks")
        _h = [None]
        mod.set_axon_ntff_profile_hook = lambda h: _h.__setitem__(0, h)
        mod.get_axon_ntff_profile_hook = lambda: _h[0]
        sys.modules["antenv.axon_hooks"] = mod
        import antenv

        antenv.axon_hooks = mod
        from trn_agent_boot.trn_boot import _ntff_profile_via_ctypes

        mod.set_axon_ntff_profile_hook(
            _ntff_profile_via_ctypes("/opt/axon/libaxon_pjrt.so"))

    import concourse.bass_utils as bu

    bu.upload_artifacts = lambda tmpdir: f"local://{tmpdir}"


def kernel(x: np.ndarray, weight: np.ndarray, bias: np.ndarray) -> np.ndarray:
    global last_results
    from concourse.bass_utils import run_bass_kernel_spmd

    x = np.asarray(x, dtype=np.float32)
    weight = np.asarray(weight, dtype=np.float32)
    bias = np.asarray(bias, dtype=np.float32)

    nc = _build_program()
    in_maps = _make_in_maps(x, weight, bias)
    trace = bool(int(os.environ.get("KERNEL_TRACE", "0")))
    trace_cores = None
    if trace:
        _setup_trace_hooks()
        tc_env = os.environ.get("KERNEL_TRACE_CORES", "")
        if tc_env:
            trace_cores = [int(c) for c in tc_env.split(",")]
    res = run_bass_kernel_spmd(nc, in_maps, list(range(N_CORES)), trace=trace,
                               trace_cores=trace_cores)
    last_results = res

    out = np.empty((B, T, O), dtype=np.float32)
    for core in range(N_CORES):
        out[core] = res.results[core]["yT"].T.astype(np.float32)
    return out


# revision 5
# speedup vs baseline: 1.2069x; 1.1636x over previous
"""BinaryLinear (binarized nn.Linear) on 8 Trainium2 NeuronCores.

Reference op:
    alpha = mean(|W|, axis=1)                # per-output-row scale
    BW    = sign(W) * alpha                  # sign(0) := +1
    Y     = einsum('bsi,oi->bso', X, BW) + bias

Distribution: data-parallel over the batch dim (8 batches -> 1 per core).
Each core receives its batch slice of X pre-transposed and cast to bf16
(xT = [in, tok]), the full weight in both layouts as bf16 (wT = [in, out]
for the matmul stationary operand, w = [out, in] for the per-row alpha
reduction), and bias f32. Each core computes the full [tok, out] output
for its batch element (stored transposed as [out, tok], bf16); the host
casts back to f32, transposes and stacks.

Numerics: binarized weights are exactly +-0.5 in bf16 (the missing x2 is
folded into alpha2 = 2*mean|W|), so the only quantization is x->bf16 and
the bf16 output store: ~0.2% rel error vs the 2e-2 gate.

On-device per core:
  - sign half-trick: s = (w >= 0) - 0.5 in {+0.5, -0.5} (one DVE op).
  - alpha: DVE abs-accumulate reduce over natural-layout bf16 weight rows
    into f32.
  - matmul: bf16 (full-rate PE + FWL weight loads), K=2048 accumulated in
    PSUM f32. Warmup: pair-0 out-chunks run with the k-chunk loop
    OUTERMOST so each arriving 512 KiB x-chunk unblocks 8 matmuls (all 8
    PSUM banks); bf16 chunk DMA (1.4us) < 8 MMs (1.8us) so the PE never
    starves once the first chunk lands.
  - DMA emission order on the in-order SP ring: pair-0 sign source first,
    then the 16 x chunks, then alphas + later pairs' weights - this puts
    the first matmul ~4us in instead of waiting on all weight prep.
  - epilogue: one ScalarE activation per psum tile:
    Identity(psum*alpha2 + bias) -> bf16, then DMA out on the ACT HW-DGE
    ring (keeps the SP ring pure loads).
"""

import os

import numpy as np

B, T, K, O = 8, 2048, 2048, 2048  # batch, tokens, in_features, out_features
P = 128          # SBUF partitions
KC = K // P      # 16 k-chunks
OC = O // P      # 16 out-chunks
TN = 512         # moving free-dim per matmul
TT = T // TN     # 4 token tiles

N_CORES = 8

# Stashed by kernel() for test harnesses: BassKernelResults of the last run.
last_results = None

_cached_nc = None


def _build_program():
    global _cached_nc
    if _cached_nc is not None:
        return _cached_nc

    import concourse.tile as tile
    from concourse import bacc, bass_isa, mybir

    F32 = mybir.dt.float32
    F32R = mybir.dt.float32r
    BF16 = mybir.dt.float16  # fp16: 16-bit like bf16 but testing PE stream rate
    IDENT = mybir.ActivationFunctionType.Identity
    ALU = mybir.AluOpType
    AX = mybir.AxisListType

    nc = bacc.Bacc("TRN2", target_bir_lowering=False, debug=False,
                   num_devices=N_CORES)

    xT = nc.dram_tensor("xT", [K, T], BF16, kind="ExternalInput").ap()
    wT = nc.dram_tensor("wT", [K, O], BF16, kind="ExternalInput").ap()
    w = nc.dram_tensor("w", [O, K], BF16, kind="ExternalInput").ap()
    b = nc.dram_tensor("b", [O], F32, kind="ExternalInput").ap()
    yT = nc.dram_tensor("yT", [O, T], BF16, kind="ExternalOutput").ap()

    xT_r = xT.rearrange("(c p) t -> p c t", p=P)
    wT_r = wT.rearrange("(c p) o -> p c o", p=P)

    with tile.TileContext(nc) as tc:
        with (
            tc.tile_pool(name="xpool", bufs=1) as xpool,
            tc.tile_pool(name="wpool", bufs=2) as wpool,
            tc.tile_pool(name="spool", bufs=4) as spool,
            tc.tile_pool(name="npool", bufs=2) as npool,
            tc.tile_pool(name="apool", bufs=6) as apool,
            tc.tile_pool(name="opool", bufs=3) as opool,
            tc.tile_pool(name="const", bufs=1) as const,
            tc.tile_pool(name="psum", bufs=8, space="PSUM") as psum,
        ):
            def sign_prep(o):
                """Load + binarize the stationary operand for out-chunk o."""
                wraw = wpool.tile([P, KC, P], BF16, tag="wraw",
                                  name=f"wraw{o}")
                nc.sync.dma_start(out=wraw, in_=wT_r[:, :, o * P:(o + 1) * P])
                sw = spool.tile([P, KC, P], BF16, tag="sw", name=f"sw{o}")
                nc.vector.tensor_scalar(sw, wraw, 0.0, 0.5,
                                        op0=ALU.is_ge, op1=ALU.subtract)
                return sw

            def alpha_prep(o):
                """alpha2 = 2*mean|W_row| from the natural-layout rows."""
                wn = npool.tile([P, K], BF16, tag="wn", name=f"wn{o}")
                nc.sync.dma_start(out=wn, in_=w[o * P:(o + 1) * P, :])
                asum = apool.tile([P, 1], F32, tag="asum", name=f"as{o}")
                nc.vector.tensor_reduce(asum, wn, axis=AX.X, op=ALU.add,
                                        apply_absolute_value=True)
                alpha2 = apool.tile([P, 1], F32, tag="alpha2", name=f"al{o}")
                nc.vector.tensor_scalar_mul(alpha2, asum, 2.0 / K)
                return alpha2

            def weight_prep(o):
                return sign_prep(o), alpha_prep(o)

            # pair-0 sign source goes ahead of the x stream (the first
            # matmuls need it); everything else queues behind x
            sw0 = sign_prep(0)
            sw1 = sign_prep(1)

            # resident x: 16 chunk tiles [128, 2048] bf16 (i on partitions)
            x_tiles = []
            bias_sb = None
            for c in range(KC):
                xt = xpool.tile([P, T], BF16, tag=f"x{c}")
                nc.sync.dma_start(out=xt, in_=xT_r[:, c, :])
                x_tiles.append(xt)
                if c == 1:
                    # bias: epilogue-only, tiny [128,16]
                    bias_sb = const.tile([P, OC], F32)
                    nc.sync.dma_start(out=bias_sb,
                                      in_=b.rearrange("(c p) -> p c", p=P))

            # alphas for pair 0 (needed by its epilogues ~30us in), then
            # pair-1 weights (needed when steady state starts)
            a0 = alpha_prep(0)
            a1 = alpha_prep(1)
            prepped = {0: (sw0, a0), 1: (sw1, a1),
                       2: weight_prep(2), 3: weight_prep(3)}

            def mm_group(ps_t, sw, t, c_lo, c_hi):
                for c in range(c_lo, c_hi):
                    nc.tensor.matmul(
                        ps_t, lhsT=sw[:, c, :],
                        rhs=x_tiles[c][:, t * TN:(t + 1) * TN],
                        start=(c == c_lo), stop=(c == c_hi - 1))

            def epilogue(ps_t, o, t, a2, name):
                ot = opool.tile([P, TN], BF16, tag="ot", name=name)
                nc.scalar.activation(ot, ps_t, IDENT,
                                     bias=bias_sb[:, o:o + 1], scale=a2)
                # issue output DMAs on the ACT HW-DGE ring: the SP ring's
                # in-order issue stream must stay pure loads, else weight
                # prefetch DMAs queue behind epilogue-gated stores
                nc.scalar.dma_start(
                    out=yT[o * P:(o + 1) * P, t * TN:(t + 1) * TN], in_=ot)

            for pair in range(OC // 2):
                o0, o1 = 2 * pair, 2 * pair + 1
                pair_w = [prepped.pop(o0), prepped.pop(o1)]
                ps = [psum.tile([P, TN], F32, tag="ps", name=f"ps{pair}_{i}")
                      for i in range(8)]

                if pair == 0:
                    # x still streaming in: k-chunk outermost so every
                    # arriving x chunk unblocks 8 matmuls (all psum banks)
                    for c in range(KC):
                        for j in range(2):
                            sw = pair_w[j][0]
                            for t in range(TT):
                                nc.tensor.matmul(
                                    ps[j * TT + t],
                                    lhsT=sw[:, c, :],
                                    rhs=x_tiles[c][:, t * TN:(t + 1) * TN],
                                    start=(c == 0),
                                    stop=(c == KC - 1),
                                )
                    for j in range(2):
                        for t in range(TT):
                            epilogue(ps[j * TT + t], (o0, o1)[j], t,
                                     pair_w[j][1], f"ot{pair}_{j}_{t}")
                else:
                    # steady state: one psum group at a time so groups finish
                    # staggered -- banks free incrementally and epilogues
                    # overlap the next group's matmuls
                    for j in range(2):
                        for t in range(TT):
                            mm_group(ps[j * TT + t], pair_w[j][0], t, 0, KC)
                            epilogue(ps[j * TT + t], (o0, o1)[j], t,
                                     pair_w[j][1], f"ot{pair}_{j}_{t}")

                # prefetch weights two pairs out (pair 0 and 1 were loaded
                # up front); emitted after this pair's matmuls so the DMAs
                # queue behind the x chunks on the in-order SP ring
                nxt = 2 * pair + 4
                if nxt < OC:
                    prepped[nxt] = weight_prep(nxt)
                    prepped[nxt + 1] = weight_prep(nxt + 1)

    nc.compile()
    _cached_nc = nc
    return nc


def _make_in_maps(x, weight, bias):
    import ml_dtypes

    bf16 = np.float16
    wT = np.ascontiguousarray(weight.T).astype(bf16)
    w = np.ascontiguousarray(weight).astype(bf16)
    b = np.ascontiguousarray(bias)
    in_maps = []
    for core in range(N_CORES):
        xb = np.ascontiguousarray(x[core].T).astype(bf16)  # [in, tok]
        in_maps.append({"xT": xb, "wT": wT, "w": w, "b": b})
    return in_maps


def _setup_trace_hooks():
    """Provide the antenv.axon_hooks NTFF hook missing from this image and
    skip the artifact bucket upload so trace=True works locally."""
    import sys
    import types

    try:
        from antenv.axon_hooks import get_axon_ntff_profile_hook  # noqa: F401
    except ImportError:
        mod = types.ModuleType("antenv.axon_hooks")
        _h = [None]
        mod.set_axon_ntff_profile_hook = lambda h: _h.__setitem__(0, h)
        mod.get_axon_ntff_profile_hook = lambda: _h[0]
        sys.modules["antenv.axon_hooks"] = mod
        import antenv

        antenv.axon_hooks = mod
        from trn_agent_boot.trn_boot import _ntff_profile_via_ctypes

        mod.set_axon_ntff_profile_hook(
            _ntff_profile_via_ctypes("/opt/axon/libaxon_pjrt.so"))

    import concourse.bass_utils as bu

    bu.upload_artifacts = lambda tmpdir: f"local://{tmpdir}"


def kernel(x: np.ndarray, weight: np.ndarray, bias: np.ndarray) -> np.ndarray:
    global last_results
    from concourse.bass_utils import run_bass_kernel_spmd

    x = np.asarray(x, dtype=np.float32)
    weight = np.asarray(weight, dtype=np.float32)
    bias = np.asarray(bias, dtype=np.float32)

    nc = _build_program()
    in_maps = _make_in_maps(x, weight, bias)
    trace = bool(int(os.environ.get("KERNEL_TRACE", "0")))
    trace_cores = None
    if trace:
        _setup_trace_hooks()
        tc_env = os.environ.get("KERNEL_TRACE_CORES", "")
        if tc_env:
            trace_cores = [int(c) for c in tc_env.split(",")]
    res = run_bass_kernel_spmd(nc, in_maps, list(range(N_CORES)), trace=trace,
                               trace_cores=trace_cores)
    last_results = res

    out = np.empty((B, T, O), dtype=np.float32)
    for core in range(N_CORES):
        out[core] = res.results[core]["yT"].T.astype(np.float32)
    return out


# revision 6
# speedup vs baseline: 1.2352x; 1.0234x over previous
"""BinaryLinear (binarized nn.Linear) on 8 Trainium2 NeuronCores.

Reference op:
    alpha = mean(|W|, axis=1)                # per-output-row scale
    BW    = sign(W) * alpha                  # sign(0) := +1
    Y     = einsum('bsi,oi->bso', X, BW) + bias

Distribution: data-parallel over the batch dim (8 batches -> 1 per core).
Each core receives its batch slice of X pre-transposed and cast to bf16
(xT = [in, tok]), the full weight in both layouts as bf16 (wT = [in, out]
for the matmul stationary operand, w = [out, in] for the per-row alpha
reduction), and bias f32. Each core computes the full [tok, out] output
for its batch element (stored transposed as [out, tok], bf16); the host
casts back to f32, transposes and stacks.

Numerics: binarized weights are exactly +-0.5 in bf16 (the missing x2 is
folded into alpha2 = 2*mean|W|), so the only quantization is x->bf16 and
the bf16 output store: ~0.2% rel error vs the 2e-2 gate.

On-device per core:
  - sign half-trick: s = (w >= 0) - 0.5 in {+0.5, -0.5} (one DVE op).
  - alpha: DVE abs-accumulate reduce over natural-layout bf16 weight rows
    into f32.
  - matmul: bf16 (full-rate PE + FWL weight loads), K=2048 accumulated in
    PSUM f32. Warmup: pair-0 out-chunks run with the k-chunk loop
    OUTERMOST so each arriving 512 KiB x-chunk unblocks 8 matmuls (all 8
    PSUM banks); bf16 chunk DMA (1.4us) < 8 MMs (1.8us) so the PE never
    starves once the first chunk lands.
  - DMA emission order on the in-order SP ring: pair-0 sign source first,
    then the 16 x chunks, then alphas + later pairs' weights - this puts
    the first matmul ~4us in instead of waiting on all weight prep.
  - epilogue: one ScalarE activation per psum tile:
    Identity(psum*alpha2 + bias) -> bf16, then DMA out on the ACT HW-DGE
    ring (keeps the SP ring pure loads).
"""

import os

import numpy as np

B, T, K, O = 8, 2048, 2048, 2048  # batch, tokens, in_features, out_features
P = 128          # SBUF partitions
KC = K // P      # 16 k-chunks
OC = O // P      # 16 out-chunks
TN = 512         # moving free-dim per matmul
TT = T // TN     # 4 token tiles

N_CORES = 8

# Stashed by kernel() for test harnesses: BassKernelResults of the last run.
last_results = None

_cached_nc = None


def _build_program():
    global _cached_nc
    if _cached_nc is not None:
        return _cached_nc

    import concourse.tile as tile
    from concourse import bacc, bass_isa, mybir

    F32 = mybir.dt.float32
    F32R = mybir.dt.float32r
    BF16 = mybir.dt.float16  # fp16: 16-bit like bf16 but testing PE stream rate
    IDENT = mybir.ActivationFunctionType.Identity
    ALU = mybir.AluOpType
    AX = mybir.AxisListType

    nc = bacc.Bacc("TRN2", target_bir_lowering=False, debug=False,
                   num_devices=N_CORES)

    xT = nc.dram_tensor("xT", [K, T], BF16, kind="ExternalInput").ap()
    # wS: host-pretiled stationary source, wS[oc, p, c*128+j] =
    # weight[oc*128+j, c*128+p] (x1024 to dodge fp16 subnormal sign loss)
    # -- each o-chunk loads as one [128, 2048] tile with 4 KiB contiguous
    # partition rows instead of 256 B strided segments
    wS = nc.dram_tensor("wS", [OC, P, K], BF16, kind="ExternalInput").ap()
    w = nc.dram_tensor("w", [O, K], BF16, kind="ExternalInput").ap()
    b = nc.dram_tensor("b", [O], F32, kind="ExternalInput").ap()
    yT = nc.dram_tensor("yT", [O, T], BF16, kind="ExternalOutput").ap()

    xT_r = xT.rearrange("(c p) t -> p c t", p=P)

    with tile.TileContext(nc) as tc:
        with (
            tc.tile_pool(name="xpool", bufs=1) as xpool,
            tc.tile_pool(name="wpool", bufs=2) as wpool,
            tc.tile_pool(name="spool", bufs=4) as spool,
            tc.tile_pool(name="npool", bufs=2) as npool,
            tc.tile_pool(name="apool", bufs=6) as apool,
            tc.tile_pool(name="opool", bufs=3) as opool,
            tc.tile_pool(name="const", bufs=1) as const,
            tc.tile_pool(name="psum", bufs=8, space="PSUM") as psum,
        ):
            def sign_prep(o):
                """Load + binarize the stationary operand for out-chunk o."""
                wraw = wpool.tile([P, K], BF16, tag="wraw",
                                  name=f"wraw{o}")
                nc.sync.dma_start(out=wraw, in_=wS[o])
                sw = spool.tile([P, KC, P], BF16, tag="sw", name=f"sw{o}")
                nc.vector.tensor_scalar(sw, wraw, 0.0, 0.5,
                                        op0=ALU.is_ge, op1=ALU.subtract)
                return sw

            def alpha_prep(o):
                """alpha2 = 2*mean|W_row| from the natural-layout rows."""
                wn = npool.tile([P, K], BF16, tag="wn", name=f"wn{o}")
                nc.sync.dma_start(out=wn, in_=w[o * P:(o + 1) * P, :])
                asum = apool.tile([P, 1], F32, tag="asum", name=f"as{o}")
                nc.vector.tensor_reduce(asum, wn, axis=AX.X, op=ALU.add,
                                        apply_absolute_value=True)
                alpha2 = apool.tile([P, 1], F32, tag="alpha2", name=f"al{o}")
                nc.vector.tensor_scalar_mul(alpha2, asum, 2.0 / K)
                return alpha2

            def weight_prep(o):
                return sign_prep(o), alpha_prep(o)

            # pair-0 sign source goes ahead of the x stream (the first
            # matmuls need it); everything else queues behind x
            sw0 = sign_prep(0)
            sw1 = sign_prep(1)

            # resident x: 16 chunk tiles [128, 2048] bf16 (i on partitions)
            x_tiles = []
            bias_sb = None
            for c in range(KC):
                xt = xpool.tile([P, T], BF16, tag=f"x{c}")
                nc.sync.dma_start(out=xt, in_=xT_r[:, c, :])
                x_tiles.append(xt)
                if c == 1:
                    # bias: epilogue-only, tiny [128,16]
                    bias_sb = const.tile([P, OC], F32)
                    nc.sync.dma_start(out=bias_sb,
                                      in_=b.rearrange("(c p) -> p c", p=P))

            # alphas for pair 0 first: they gate pair-0 epilogues, which
            # free the psum banks pair-1's first matmuls need; pair-1 sign
            # sources right behind, alphas after
            a0 = alpha_prep(0)
            a1 = alpha_prep(1)
            sw2 = sign_prep(2)
            sw3 = sign_prep(3)
            prepped = {0: (sw0, a0), 1: (sw1, a1),
                       2: (sw2, alpha_prep(2)), 3: (sw3, alpha_prep(3))}

            def mm_group(ps_t, sw, t, c_lo, c_hi):
                for c in range(c_lo, c_hi):
                    nc.tensor.matmul(
                        ps_t, lhsT=sw[:, c, :],
                        rhs=x_tiles[c][:, t * TN:(t + 1) * TN],
                        start=(c == c_lo), stop=(c == c_hi - 1))

            def epilogue(ps_t, o, t, a2, name):
                ot = opool.tile([P, TN], BF16, tag="ot", name=name)
                nc.scalar.activation(ot, ps_t, IDENT,
                                     bias=bias_sb[:, o:o + 1], scale=a2)
                # issue output DMAs on the ACT HW-DGE ring: the SP ring's
                # in-order issue stream must stay pure loads, else weight
                # prefetch DMAs queue behind epilogue-gated stores
                nc.scalar.dma_start(
                    out=yT[o * P:(o + 1) * P, t * TN:(t + 1) * TN], in_=ot)

            for pair in range(OC // 2):
                o0, o1 = 2 * pair, 2 * pair + 1
                pair_w = [prepped.pop(o0), prepped.pop(o1)]
                ps = [psum.tile([P, TN], F32, tag="ps", name=f"ps{pair}_{i}")
                      for i in range(8)]

                if pair == 0:
                    # x still streaming in: k-chunk outermost so every
                    # arriving x chunk unblocks 8 matmuls (all psum banks)
                    for c in range(KC):
                        for j in range(2):
                            sw = pair_w[j][0]
                            for t in range(TT):
                                nc.tensor.matmul(
                                    ps[j * TT + t],
                                    lhsT=sw[:, c, :],
                                    rhs=x_tiles[c][:, t * TN:(t + 1) * TN],
                                    start=(c == 0),
                                    stop=(c == KC - 1),
                                )
                    for j in range(2):
                        for t in range(TT):
                            epilogue(ps[j * TT + t], (o0, o1)[j], t,
                                     pair_w[j][1], f"ot{pair}_{j}_{t}")
                else:
                    # steady state: one psum group at a time so groups finish
                    # staggered -- banks free incrementally and epilogues
                    # overlap the next group's matmuls
                    for j in range(2):
                        for t in range(TT):
                            mm_group(ps[j * TT + t], pair_w[j][0], t, 0, KC)
                            epilogue(ps[j * TT + t], (o0, o1)[j], t,
                                     pair_w[j][1], f"ot{pair}_{j}_{t}")

                # prefetch weights two pairs out (pair 0 and 1 were loaded
                # up front); emitted after this pair's matmuls so the DMAs
                # queue behind the x chunks on the in-order SP ring
                nxt = 2 * pair + 4
                if nxt < OC:
                    prepped[nxt] = weight_prep(nxt)
                    prepped[nxt + 1] = weight_prep(nxt + 1)

    nc.compile()
    _cached_nc = nc
    return nc


def _make_in_maps(x, weight, bias):
    f16 = np.float16
    # pretiled stationary source: wS[oc, p, c*128+j] = weight[oc*128+j,
    # c*128+p], scaled x1024 so near-zero weights keep their sign in fp16
    # (only the sign is consumed); alpha comes from the unscaled copy w
    wS = np.ascontiguousarray(
        (weight * 1024.0).reshape(OC, P, KC, P).transpose(0, 3, 2, 1)
        .reshape(OC, P, K)).astype(f16)
    w = np.ascontiguousarray(weight).astype(f16)
    b = np.ascontiguousarray(bias)
    in_maps = []
    for core in range(N_CORES):
        xb = np.ascontiguousarray(x[core].T).astype(f16)  # [in, tok]
        in_maps.append({"xT": xb, "wS": wS, "w": w, "b": b})
    return in_maps


def _setup_trace_hooks():
    """Provide the antenv.axon_hooks NTFF hook missing from this image and
    skip the artifact bucket upload so trace=True works locally."""
    import sys
    import types

    try:
        from antenv.axon_hooks import get_axon_ntff_profile_hook  # noqa: F401
    except ImportError:
        mod = types.ModuleType("antenv.axon_hooks")
        _h = [None]
        mod.set_axon_ntff_profile_hook = lambda h: _h.__setitem__(0, h)
        mod.get_axon_ntff_profile_hook = lambda: _h[0]
        sys.modules["antenv.axon_hooks"] = mod
        import antenv

        antenv.axon_hooks = mod
        from trn_agent_boot.trn_boot import _ntff_profile_via_ctypes

        mod.set_axon_ntff_profile_hook(
            _ntff_profile_via_ctypes("/opt/axon/libaxon_pjrt.so"))

    import concourse.bass_utils as bu

    bu.upload_artifacts = lambda tmpdir: f"local://{tmpdir}"


def kernel(x: np.ndarray, weight: np.ndarray, bias: np.ndarray) -> np.ndarray:
    global last_results
    from concourse.bass_utils import run_bass_kernel_spmd

    x = np.asarray(x, dtype=np.float32)
    weight = np.asarray(weight, dtype=np.float32)
    bias = np.asarray(bias, dtype=np.float32)

    nc = _build_program()
    in_maps = _make_in_maps(x, weight, bias)
    trace = bool(int(os.environ.get("KERNEL_TRACE", "0")))
    trace_cores = None
    if trace:
        _setup_trace_hooks()
        tc_env = os.environ.get("KERNEL_TRACE_CORES", "")
        if tc_env:
            trace_cores = [int(c) for c in tc_env.split(",")]
    res = run_bass_kernel_spmd(nc, in_maps, list(range(N_CORES)), trace=trace,
                               trace_cores=trace_cores)
    last_results = res

    out = np.empty((B, T, O), dtype=np.float32)
    for core in range(N_CORES):
        out[core] = res.results[core]["yT"].T.astype(np.float32)
    return out
